# revision 18
# baseline (speedup 1.0000x reference)
"""Locally-connected graph-conv kernel for Trainium2 (Bass/Tile).

Computes out[b,t,m] = sum_n x[b,t,n] * (S*W)[n,m] + bias[m] for
x [64, 2048, 208], W/S [208, 208], bias [208].

The ring-graph support S is a +-4 band (mod 208), so each half of the
output nodes only needs a 112-row slice of the contraction dim (halo
included). The host pre-assembles, per 8-core data-parallel shard, a
[112, 2*SHARD] tensor whose left half (per time-chunk) is block 0's
rotated rows and right half block 1's:
  block 0 (m 0..103):   x nodes {204..207, 0..107}
  block 1 (m 104..207): x nodes {100..207, 0..3}
Each output block is then a SINGLE [112,104] x [112,512] matmul with
the host-premasked weight block stationary in the PE array.

Everything that touches HBM is bf16 (PSUM accumulation stays fp32): the
2e-2 rel-err budget dwarfs bf16 rounding (~5e-3) and it halves DMA bytes
vs fp32. HBM per NeuronCore is ~358 GB/s per direction (ramping up over
the first ~20 us), so the ~14.9 MB/core of traffic floors the kernel at
~45 us. Scheduling lessons baked in:
 - one load DMA and one store DMA per chunk ([112, 2*csz], 8-16 KB
   descriptors): big DMAs run at ~340-420 GB/s vs ~245 for small ones,
   and every DMA/tile costs extra issue time plus ~90 ns/semaphore in
   the framework's exit sem-clear loop.
 - all x loads ride the Sync ring in consumption order, issued up-front
   into persistent SBUF tiles (x fits in SBUF; no WARs, so the stream
   runs at whatever rate HBM gives). All stores ride the Scalar ring.
 - weights + bias are padded to >=1KB DMA rows (tiny-descriptor DMAs
   crawl at ~27 GB/s and once clogged a ring for 11 us) and ride the
   Scalar ring ahead of the stores; the GpSimd SWDGE path is avoided
   (its completion semaphore fires ~7 us late).
 - PSUM->SBUF eviction is stuck at 1 elem/lane/cycle (fp32 PSUM source),
   so block 0 evicts on VectorE (tensor_scalar add) and block 1 on
   ScalarE (Identity activation with bias AP) concurrently.
 - a few dummy matmuls on the weight tile right after it lands warm the
   PE HAM clock gate (cold 1.2 GHz -> warm 2.4 GHz) before data arrives.
The host transposes/de-interleaves y^T back at gather.
"""

import numpy as np
import ml_dtypes
from contextlib import ExitStack

import concourse.bacc as bacc
import concourse.mybir as mybir
import concourse.tile as tile
from concourse.bass_utils import run_bass_kernel_spmd

N = 208                      # nodes
HALF = 104                   # output nodes per block
K = 4                        # band half-width of S
NH = 2 * K + HALF            # 112 contraction rows per block (halo incl.)
WPAD = 1024                  # wh DRAM row padding (2 KB rows -> fast DMA)
BPAD = 256                   # bias DRAM row padding (1 KB f32 rows)
N_CORES = 8
B, T = 64, 2048
ROWS_TOTAL = B * T           # 131072
SHARD = ROWS_TOTAL // N_CORES    # 16384 rows per core
TB = 512                     # moving-block columns per matmul (fp32 PSUM max)
TB2 = 2 * TB                 # eviction group (2 PSUM banks)
CHUNKS = [2048, 2048, 4096, 4096, 2048, 2048]   # t-cols per pipeline chunk
assert sum(CHUNKS) == SHARD
N_DUMMY = 4                  # PE warm-up matmuls on the weight tile

FP32 = mybir.dt.float32
BF16 = mybir.dt.bfloat16
NP_BF16 = ml_dtypes.bfloat16
IDENT = mybir.ActivationFunctionType.Identity

# block contraction rows (indices into the [208] node dim)
ROWS0 = list(range(N - K, N)) + list(range(0, HALF + K))          # 112
ROWS1 = list(range(HALF - K, N)) + list(range(0, K))              # 112

_CACHE = {}
LAST_RESULTS = None          # BassKernelResults of the most recent run


def _kernel_body(tc):
    nc = tc.nc
    # per chunk c: cols [2*col, 2*col+csz) = block0 rows, [.. +csz) = block1
    x_d = nc.dram_tensor("xh", [NH, 2 * SHARD], BF16, kind="ExternalInput").ap()
    w_d = nc.dram_tensor("wh", [NH, WPAD], BF16, kind="ExternalInput").ap()
    b_d = nc.dram_tensor("bias", [2 * NH, BPAD], FP32, kind="ExternalInput").ap()
    o_d = nc.dram_tensor("outt", [NH, 2 * SHARD], BF16, kind="ExternalOutput").ap()

    with ExitStack() as ctx:
        const = ctx.enter_context(tc.tile_pool(name="const", bufs=1))

        # One-time setup on the Scalar ring (it carries no loads):
        # weights first (the warm-up matmuls need them), then bias.
        wh = const.tile([NH, WPAD], BF16, tag="wh")
        nc.scalar.dma_start(wh, w_d)
        bA = const.tile([NH, BPAD], FP32, tag="bA")
        bB = const.tile([NH, BPAD], FP32, tag="bB")
        nc.scalar.dma_start(bA, b_d[0:NH, :])
        nc.scalar.dma_start(bB, b_d[NH : 2 * NH, :])
        bAc = bA[0:HALF, 0:1]
        bBc = bB[0:HALF, 0:1]

        ps0p = ctx.enter_context(tc.tile_pool(name="ps0p", bufs=2, space="PSUM"))
        ps1p = ctx.enter_context(tc.tile_pool(name="ps1p", bufs=2, space="PSUM"))

        # All x loads up-front on the Sync ring, one DMA per chunk, into
        # persistent tiles. Output tiles are persistent too, so no
        # eviction ever waits on a store and no store on a tile WAR.
        xts = []
        col = 0
        for c, csz in enumerate(CHUNKS):
            xt = const.tile([NH, 2 * csz], BF16, tag=f"x_{c}")
            nc.sync.dma_start(xt, x_d[:, 2 * col : 2 * (col + csz)])
            xts.append((xt, col, csz))
            col += csz

        # PE warm-up: HAM un-throttles (1.2 -> 2.4 GHz) after ~3.4us of
        # sustained busy; burn idle pre-data time on the weight tile.
        for _ in range(N_DUMMY):
            psd = ps0p.tile([HALF, TB2], FP32, tag="ps0")
            nc.tensor.matmul(psd[:, 0:TB], wh[:, 0:HALF], wh[:, 0:TB], start=True, stop=True)

        for c, (xt, col, csz) in enumerate(xts):
            o_t = const.tile([NH, 2 * csz], BF16, tag=f"o_{c}")
            for s in range(csz // TB2):
                g = slice(s * TB2, (s + 1) * TB2)
                ga = slice(s * TB2, s * TB2 + TB)
                gb = slice(s * TB2 + TB, (s + 1) * TB2)
                g1 = slice(csz + s * TB2, csz + (s + 1) * TB2)
                ga1 = slice(csz + s * TB2, csz + s * TB2 + TB)
                gb1 = slice(csz + s * TB2 + TB, csz + (s + 1) * TB2)
                # [104, 1024] PSUM tiles (2 banks); one matmul per bank
                ps0 = ps0p.tile([HALF, TB2], FP32, tag="ps0")
                nc.tensor.matmul(ps0[:, 0:TB], wh[:, 0:HALF], xt[:, ga], start=True, stop=True)
                nc.tensor.matmul(ps0[:, TB:TB2], wh[:, 0:HALF], xt[:, gb], start=True, stop=True)
                ps1 = ps1p.tile([HALF, TB2], FP32, tag="ps1")
                nc.tensor.matmul(ps1[:, 0:TB], wh[:, HALF:N], xt[:, ga1], start=True, stop=True)
                nc.tensor.matmul(ps1[:, TB:TB2], wh[:, HALF:N], xt[:, gb1], start=True, stop=True)
                # evictions split across engines; both fuse bias + fp32->bf16
                nc.vector.tensor_scalar_add(o_t[0:HALF, g], ps0, bAc)
                nc.scalar.activation(o_t[0:HALF, g1], ps1, IDENT, bias=bBc)
            # one store per chunk on the Scalar ring
            nc.scalar.dma_start(o_d[:, 2 * col : 2 * (col + csz)], o_t)


def _build():
    nc = bacc.Bacc(
        "TRN2",
        target_bir_lowering=False,
        debug=False,
        num_devices=N_CORES,
    )
    with tile.TileContext(nc) as tc:
        _kernel_body(tc)
    nc.compile()
    return nc


def kernel(x, W, b, S):
    global LAST_RESULTS
    nc = _CACHE.get("nc")
    if nc is None:
        nc = _build()
        _CACHE["nc"] = nc

    xf = np.asarray(x, np.float32).reshape(ROWS_TOTAL, N)
    SW = (np.asarray(S, np.float32) * np.asarray(W, np.float32))
    wh = np.zeros((NH, WPAD), NP_BF16)
    wh[:, 0:HALF] = SW[ROWS0, 0:HALF]
    wh[:, HALF:N] = SW[ROWS1, HALF:N]
    bfv = np.asarray(b, np.float32).reshape(N)
    bf = np.zeros((2 * NH, BPAD), np.float32)
    bf[0:HALF, 0] = bfv[0:HALF]
    bf[NH : NH + HALF, 0] = bfv[HALF:N]

    in_maps = []
    for i in range(N_CORES):
        xt = xf[i * SHARD : (i + 1) * SHARD].T          # [208, SHARD] view
        xb = np.asarray(xt, NP_BF16)
        xh = np.empty((NH, 2 * SHARD), NP_BF16)
        col = 0
        for csz in CHUNKS:
            seg = slice(col, col + csz)
            dst0 = slice(2 * col, 2 * col + csz)
            dst1 = slice(2 * col + csz, 2 * (col + csz))
            xh[0:K, dst0] = xb[N - K : N, seg]
            xh[K:NH, dst0] = xb[0 : HALF + K, seg]
            xh[0 : NH - K, dst1] = xb[HALF - K : N, seg]
            xh[NH - K : NH, dst1] = xb[0:K, seg]
            col += csz
        in_maps.append({"xh": xh, "wh": wh, "bias": bf})
    res = run_bass_kernel_spmd(nc, in_maps, core_ids=list(range(N_CORES)))
    LAST_RESULTS = res
    out = np.empty((ROWS_TOTAL, N), np.float32)
    for i, r in enumerate(res.results):
        yt = r["outt"]                                  # [112, 2*SHARD] bf16
        sl = slice(i * SHARD, (i + 1) * SHARD)
        col = 0
        for csz in CHUNKS:
            seg = slice(i * SHARD + col, i * SHARD + col + csz)
            out[seg, 0:HALF] = yt[0:HALF, 2 * col : 2 * col + csz].T
            out[seg, HALF:N] = yt[0:HALF, 2 * col + csz : 2 * (col + csz)].T
            col += csz
    return out.reshape(B, T, N)
